# revision 1
# baseline (speedup 1.0000x reference)
"""CrossAttention Trainium2 Bass kernel.

Problem (hardcoded): B=16, Lq=Lk=2048, Dq=768, Dk=1024, fp32.
  q = query @ Wq + bq ; k = key @ Wk + bk ; v = key @ Wv + bv
  out = softmax(q k^T / sqrt(1024)) @ v

Sharding: data-parallel over batch, 2 batches per core on 8 cores.

Math simplifications (exact up to fp32 rounding):
  - bk shifts every score row by a constant (per query) -> cancels in softmax,
    so bk is dropped entirely.
  - softmax weights sum to 1, so bv passes through attention unchanged:
    add bv once to the final output instead of to v.
  - scores are bounded (|s|/32 < ~3) so exp() without max-subtraction is safe.

Per-core schedule (per batch):
  A) queryT via PE transposes; qT = Wq^T queryT (+bq) ; spill qT to DRAM.
  B1) keyT via PE transposes; kT = Wk^T keyT (SBUF resident); spill keyT.
  B2) v = keyT^T Wv (SBUF resident), streaming keyT back from DRAM.
  C) flash-style attention over Lq tiles of 256:
     scoresT = kT_chunk^T qT_tile (PSUM, 8 k-chunks), expT = exp(scores/32),
     out = sum_lk expT^T v (+ones-column trick for row sums via a separate
     N=1 matmul), normalize by reciprocal of sums, + bv, DMA out.

Matmul dtype: float32r (fp32 data, fast PE mode) by default; MM_DT knob
falls back to plain float32 if hardware numerics are insufficient.
"""

import os
import numpy as np

B, LQ, LK = 16, 2048, 2048
DQ, DK = 768, 1024
N_CORES = 8
BPC = B // N_CORES  # batches per core

MM_DT = os.environ.get("XATTN_MM_DT", "float32r")


def build_nc(bpc=BPC, lq=LQ, lk=LK, mm_dt=MM_DT, lq_t=256, c_t=512, reps=1):
    import concourse.bass as bass
    import concourse.mybir as mybir
    from concourse import bacc
    import concourse.tile as tile
    from concourse.masks import make_identity

    fp32 = mybir.dt.float32
    mdt = getattr(mybir.dt, mm_dt)
    KCQ = DQ // 128   # 6 contraction chunks for q projection
    KCK = DK // 128   # 8 contraction chunks for k/v projection + scores
    NLQ = lq // lq_t  # Lq tiles (projection phase)
    NLK = lk // 128   # Lk subtiles of 128
    LS = lq_t // 128  # Lq subtiles per tile (projection phase)
    NCQ = lq // c_t   # Lq tiles (attention phase)
    CS = c_t // 128   # Lq subtiles per attention tile

    nc = bacc.Bacc("TRN2")
    query = nc.dram_tensor("query", [bpc, lq, DQ], mdt, kind="ExternalInput")
    key = nc.dram_tensor("key", [bpc, lk, DK], mdt, kind="ExternalInput")
    Wq = nc.dram_tensor("Wq", [DQ, DK], mdt, kind="ExternalInput")
    bq = nc.dram_tensor("bq", [DK], fp32, kind="ExternalInput")
    Wk = nc.dram_tensor("Wk", [DK, DK], mdt, kind="ExternalInput")
    Wv = nc.dram_tensor("Wv", [DK, DK], mdt, kind="ExternalInput")
    bv = nc.dram_tensor("bv", [DK], fp32, kind="ExternalInput")
    out = nc.dram_tensor("out", [bpc, lq, DK], fp32, kind="ExternalOutput")
    vtag = nc.dram_tensor("variant_tag", [max(1, reps), 8], fp32, kind="ExternalInput")
    qT_dram = nc.dram_tensor("qT_scratch", [bpc, 128, KCK, lq], mdt, kind="Internal")
    keyT_dram = nc.dram_tensor("keyT_scratch", [bpc, 128, KCK, lk], mdt, kind="Internal")

    def mm(ps, lhsT, rhs, start, stop):
        nc.tensor.matmul(ps, lhsT, rhs, start=start, stop=stop)

    with tile.TileContext(nc) as tc:
        with (
            tc.tile_pool(name="const", bufs=1) as constp,
            tc.tile_pool(name="kT", bufs=1) as kTp,
            tc.tile_pool(name="v", bufs=1) as vp,
        ):
            ident_f32 = constp.tile([128, 128], fp32)
            make_identity(nc, ident_f32)
            if mdt == fp32:
                ident = ident_f32
            else:
                ident = constp.tile([128, 128], mdt)
                nc.vector.tensor_copy(ident, ident_f32)
            ones_col = constp.tile([128, 4], mdt)
            if mdt == fp32:
                nc.vector.memset(ones_col, 1.0)
            else:
                ones_f32 = constp.tile([128, 4], fp32)
                nc.vector.memset(ones_f32, 1.0)
                nc.vector.tensor_copy(ones_col, ones_f32)
            bq_sb = constp.tile([128, KCK], fp32)
            nc.sync.dma_start(bq_sb, bq.rearrange("(c p) -> p c", p=128))
            bv_rep = constp.tile([128, DK], fp32)
            nc.sync.dma_start(bv_rep, bv[None, :].partition_broadcast(128))
            vt_sb = constp.tile([1, 8], fp32)
            nc.sync.dma_start(vt_sb, vtag[0:1, :])

            for b in [bb for _ in range(reps) for bb in range(bpc)]:
                kT_sb = kTp.tile([128, KCK, lk], mdt)   # kT[dk, lk]
                v_sb = vp.tile([128, NLK, DK], mdt)     # v[lk, dk]

                # ---- Phase A: qT = Wq^T queryT + bq, spilled to DRAM ----
                with (
                    tc.tile_pool(name="qproj", bufs=2) as qp,
                    tc.tile_pool(name="wq", bufs=1) as wqp,
                    tc.tile_pool(name="qps", bufs=2, space="PSUM") as qps,
                ):
                    wq_sb = wqp.tile([128, KCQ, DK], mdt)
                    nc.sync.dma_start(wq_sb, Wq.rearrange("(c p) n -> p c n", p=128))
                    for t in range(NLQ):
                        qn = qp.tile([128, LS, DQ], mdt, tag="qnat")
                        nc.sync.dma_start(
                            qn,
                            query[b, t * lq_t:(t + 1) * lq_t, :].rearrange(
                                "(s p) d -> p s d", p=128
                            ),
                        )
                        qTt = qp.tile([128, KCQ, lq_t], mdt, tag="qTt")
                        for s in range(LS):
                            for kc in range(KCQ):
                                ps = qps.tile([128, 128], mdt, tag="tp")
                                nc.tensor.transpose(
                                    ps, qn[:, s, kc * 128:(kc + 1) * 128], ident
                                )
                                nc.vector.tensor_copy(
                                    qTt[:, kc, s * 128:(s + 1) * 128], ps
                                )
                        qTsb = qp.tile([128, KCK, lq_t], mdt, tag="qTsb")
                        for mc in range(KCK):
                            ps = qps.tile([128, lq_t], fp32, tag="mm")
                            for kc in range(KCQ):
                                mm(ps, wq_sb[:, kc, mc * 128:(mc + 1) * 128],
                                   qTt[:, kc, :], kc == 0, kc == KCQ - 1)
                            nc.vector.tensor_scalar_add(
                                qTsb[:, mc, :], ps, bq_sb[:, mc:mc + 1]
                            )
                        nc.sync.dma_start(
                            qT_dram[b, :, :, t * lq_t:(t + 1) * lq_t], qTsb
                        )

                # ---- Phase B1: keyT (spill) + kT resident ----
                with (
                    tc.tile_pool(name="kproj", bufs=1) as kp,
                    tc.tile_pool(name="wk", bufs=1) as wkp,
                    tc.tile_pool(name="kps", bufs=2, space="PSUM") as kps,
                ):
                    wk_sb = wkp.tile([128, KCK, DK], mdt)
                    nc.sync.dma_start(wk_sb, Wk.rearrange("(c p) n -> p c n", p=128))
                    for t in range(lk // 512):
                        kn = kp.tile([128, 4, DK], mdt, tag="knat")
                        nc.sync.dma_start(
                            kn,
                            key[b, t * 512:(t + 1) * 512, :].rearrange(
                                "(s p) d -> p s d", p=128
                            ),
                        )
                        kTt = kp.tile([128, KCK, 512], mdt, tag="kTt")
                        for s in range(4):
                            for kc in range(KCK):
                                ps = kps.tile([128, 128], mdt, tag="tp")
                                nc.tensor.transpose(
                                    ps, kn[:, s, kc * 128:(kc + 1) * 128], ident
                                )
                                nc.vector.tensor_copy(
                                    kTt[:, kc, s * 128:(s + 1) * 128], ps
                                )
                        nc.sync.dma_start(
                            keyT_dram[b, :, :, t * 512:(t + 1) * 512], kTt
                        )
                        for mc in range(KCK):
                            ps = kps.tile([128, 512], fp32, tag="mm")
                            for kc in range(KCK):
                                mm(ps, wk_sb[:, kc, mc * 128:(mc + 1) * 128],
                                   kTt[:, kc, :], kc == 0, kc == KCK - 1)
                            nc.vector.tensor_copy(
                                kT_sb[:, mc, t * 512:(t + 1) * 512], ps
                            )

                # ---- Phase B2: v = keyT^T Wv resident ----
                with (
                    tc.tile_pool(name="vproj", bufs=2) as v2p,
                    tc.tile_pool(name="wv", bufs=1) as wvp,
                    tc.tile_pool(name="vps", bufs=2, space="PSUM") as vps,
                ):
                    wv_sb = wvp.tile([128, KCK, DK], mdt)
                    nc.sync.dma_start(wv_sb, Wv.rearrange("(c p) n -> p c n", p=128))
                    for t in range(lk // 512):
                        kTt = v2p.tile([128, KCK, 512], mdt, tag="kTt2")
                        nc.sync.dma_start(
                            kTt, keyT_dram[b, :, :, t * 512:(t + 1) * 512]
                        )
                        for s in range(4):
                            for dk in range(2):
                                ps = vps.tile([128, 512], fp32, tag="vmm")
                                for kc in range(KCK):
                                    mm(ps, kTt[:, kc, s * 128:(s + 1) * 128],
                                       wv_sb[:, kc, dk * 512:(dk + 1) * 512],
                                       kc == 0, kc == KCK - 1)
                                nc.vector.tensor_copy(
                                    v_sb[:, t * 4 + s, dk * 512:(dk + 1) * 512], ps
                                )

                # ---- Phase C: attention ----
                with (
                    tc.tile_pool(name="attn", bufs=1) as cp,
                    tc.tile_pool(name="expp", bufs=NLK + 2) as ep,
                    tc.tile_pool(name="cps_s", bufs=2, space="PSUM") as cps_s,
                    tc.tile_pool(name="cps_o", bufs=2, space="PSUM") as cps_o,
                    tc.tile_pool(name="cps_n", bufs=2, space="PSUM") as cps_n,
                ):
                    for t in range(NCQ):
                        qTs = cp.tile([128, KCK, c_t], mdt, tag="qTs")
                        nc.sync.dma_start(
                            qTs, qT_dram[b, :, :, t * c_t:(t + 1) * c_t]
                        )
                        exps = []
                        for lkb in range(NLK):
                            ps_s = cps_s.tile([128, c_t], fp32, tag="sc")
                            for kc in range(KCK):
                                mm(ps_s, kT_sb[:, kc, lkb * 128:(lkb + 1) * 128],
                                   qTs[:, kc, :], kc == 0, kc == KCK - 1)
                            ex = ep.tile([128, c_t], mdt, tag="exp")
                            nc.scalar.activation(
                                ex, ps_s, mybir.ActivationFunctionType.Exp,
                                scale=1.0 / 32.0,
                            )
                            exps.append(ex)
                        for s in range(CS):
                            ps_o = cps_o.tile([128, DK], fp32, tag="pv")
                            ps_n = cps_n.tile([128, 4], fp32, tag="sum")
                            for lkb in range(NLK):
                                lhs = exps[lkb][:, s * 128:(s + 1) * 128]
                                for dk in range(2):
                                    mm(ps_o[:, dk * 512:(dk + 1) * 512], lhs,
                                       v_sb[:, lkb, dk * 512:(dk + 1) * 512],
                                       lkb == 0, lkb == NLK - 1)
                                mm(ps_n, lhs, ones_col, lkb == 0, lkb == NLK - 1)
                            rec = cp.tile([128, 1], fp32, tag="rec")
                            nc.vector.reciprocal(rec, ps_n[:, 0:1])
                            o_sb = cp.tile([128, DK], fp32, tag="osb")
                            nc.scalar.activation(
                                o_sb, ps_o,
                                mybir.ActivationFunctionType.Copy, scale=rec,
                            )
                            nc.vector.tensor_add(o_sb, o_sb, bv_rep)
                            nc.sync.dma_start(
                                out[b, t * c_t + s * 128: t * c_t + (s + 1) * 128, :],
                                o_sb,
                            )
    return nc


_NC_CACHE = {}


def _get_nc(key=("full",)):
    if key not in _NC_CACHE:
        _NC_CACHE[key] = build_nc()
    return _NC_CACHE[key]


def kernel(**inputs):
    from concourse.bass_utils import run_bass_kernel_spmd

    f32c = lambda x: np.ascontiguousarray(np.asarray(x), dtype=np.float32)
    query = f32c(inputs["query"])
    key = f32c(inputs["key"])
    shared = {n: f32c(inputs[n]) for n in ("Wq", "bq", "Wk", "Wv", "bv")}

    nc = _get_nc()
    if not nc.is_finalized():
        nc.finalize()
    in_maps = []
    for c in range(N_CORES):
        m = dict(shared)
        m["query"] = query[c * BPC:(c + 1) * BPC]
        m["key"] = key[c * BPC:(c + 1) * BPC]
        m["variant_tag"] = np.zeros((1, 8), np.float32)
        in_maps.append(m)

    res = run_bass_kernel_spmd(nc, in_maps, core_ids=list(range(N_CORES)))
    return np.concatenate([r["out"] for r in res.results], axis=0)

